# revision 20
# baseline (speedup 1.0000x reference)
import numpy as np

_CACHE = {}

N_CORES = 8
TOK = 16384
TOK_PER = TOK // N_CORES  # 2048 tokens per core
DIM = 2048
NE = 64
TOPK = 8
KC = 128            # contraction chunk (partition dim)
NK = DIM // KC      # 16 chunks
NT = 512            # token tile = one f32 PSUM bank
NJ = TOK_PER // NT  # 4 token tiles
N_WARM = 20         # PE warm-up matmuls issued before data arrives

USE_BF16 = True
# threshold on adjacent top-(K+1) logit gaps below which routing is
# re-derived from exact fp64 logits (device noise: bf16 ~2e-3, f32r ~6e-5)
AMB_THRESH = 2e-2 if USE_BF16 else 1e-3


def _build():
    import concourse.bass as bass
    import concourse.tile as tile
    from concourse import bacc, mybir

    f32 = mybir.dt.float32
    dt_in = mybir.dt.bfloat16 if USE_BF16 else mybir.dt.float32r

    nc = bacc.Bacc(
        "TRN2",
        target_bir_lowering=False,
        debug=False,
        enable_asserts=False,
        num_devices=N_CORES,
    )
    # x host-packed as [pair, 128, 2*TOK_PER]: 8KB contiguous per partition
    # line per 1MB DMA (best measured descriptor shape)
    xP = nc.dram_tensor(
        "XP", (NK // 2, KC, 2 * TOK_PER), dt_in, kind="ExternalInput"
    ).ap()
    wP = nc.dram_tensor("WP", (KC, NK * NE), dt_in, kind="ExternalInput").ap()
    f16 = mybir.dt.float16
    out = nc.dram_tensor("logitsP", (KC, NJ * NT // 2), f16, kind="ExternalOutput").ap()

    with tile.TileContext(nc) as tc:
        with (
            tc.tile_pool(name="xpool", bufs=NK - 1) as xpool,
            tc.tile_pool(name="xlpool", bufs=NJ) as xlpool,
            tc.tile_pool(name="wpool", bufs=1) as wpool,
            tc.tile_pool(name="opool", bufs=1) as opool,
            tc.tile_pool(name="psum", bufs=2, space=bass.MemorySpace.PSUM) as psum,
        ):
            # PE warm-up: release the HAM clock throttle before data lands
            wscr = wpool.tile([KC, NT], dt_in, tag="wscr")
            nc.vector.memset(wscr[:], 0)
            pscr = psum.tile([KC, NT], f32, tag="pscr")
            for _ in range(N_WARM):
                nc.tensor.matmul(
                    pscr[:], wscr[:, 0:KC], wscr[:], start=True, stop=True,
                    skip_group_check=True,
                )

            wt = wpool.tile([KC, NK * NE], dt_in, tag="wt")
            nc.sync.dma_start(wt[:], wP)
            # chunk pairs 0..6 as 1MB DMAs; chunk 14 alone; chunk 15 as 4
            # token slices so the final matmuls pipeline with the stream tail
            xps = []
            for g in range(NK // 2 - 1):
                xp = xpool.tile([KC, 2 * TOK_PER], dt_in, tag="xp")
                nc.scalar.dma_start(xp[:], xP[g])
                xps.append(xp)
            x14 = xpool.tile([KC, TOK_PER], dt_in, tag="x14")
            nc.scalar.dma_start(x14[:], xP[NK // 2 - 1, :, 0:TOK_PER])
            xlast = []
            for t in range(NJ):
                xt = xlpool.tile([KC, NT], dt_in, tag="xl")
                nc.scalar.dma_start(
                    xt[:], xP[NK // 2 - 1, :, TOK_PER + t * NT:TOK_PER + (t + 1) * NT]
                )
                xlast.append(xt)

            # accp[jj][ph*64+e, t'] = logits[e, (2*jj+ph)*512+t']
            accp = [
                psum.tile([KC, NT], f32, tag=f"acc{jj}", name=f"acc{jj}")
                for jj in range(NJ // 2)
            ]
            for k in range(NK):
                lhsT = wt[:, k * NE:(k + 1) * NE]
                for t in range(NJ):
                    if k < NK - 2:
                        rhs = xps[k // 2][
                            :, (k % 2) * TOK_PER + t * NT:(k % 2) * TOK_PER + (t + 1) * NT
                        ]
                    elif k == NK - 2:
                        rhs = x14[:, t * NT:(t + 1) * NT]
                    else:
                        rhs = xlast[t][:]
                    jj, ph = t // 2, t % 2
                    nc.tensor.matmul(
                        accp[jj][ph * NE:(ph + 1) * NE, :],
                        lhsT,
                        rhs,
                        start=(k == 0),
                        stop=(k == NK - 1),
                    )
                # filler matmuls into the scratch bank between DMA-paced
                # bursts keep the PE HAM clock unthrottled for the tail
                if k % 2 == 1 and k < NK - 2:
                    for _ in range(2):
                        nc.tensor.matmul(
                            pscr[:], wscr[:, 0:KC], wscr[:],
                            start=True, stop=True, skip_group_check=True,
                        )
            # scaled fp16 output: 0.25*logits keeps |values| ~<1.5 so fp16
            # ulp error stays ~5e-4; host multiplies back by 4
            ot = opool.tile([KC, NJ * NT // 2], f16, tag="ot")
            for jj in range(NJ // 2):
                nc.vector.tensor_scalar_mul(ot[:, jj * NT:(jj + 1) * NT], accp[jj][:], 0.25)
                nc.scalar.dma_start(
                    out[:, jj * NT:(jj + 1) * NT], ot[:, jj * NT:(jj + 1) * NT]
                )
    nc.compile()
    return nc


def _cast_in(a):
    if USE_BF16:
        import ml_dtypes

        return np.ascontiguousarray(a.astype(ml_dtypes.bfloat16))
    return np.ascontiguousarray(a.astype(np.float32))


def prepare_in_maps(x, W):
    x = np.asarray(x, dtype=np.float32)
    W = np.asarray(W, dtype=np.float32)
    # WP[p, k*64+e] = W[e, k*128+p]
    WP = _cast_in(W.T.reshape(NK, KC, NE).transpose(1, 0, 2).reshape(KC, NK * NE))
    in_maps = []
    for i in range(N_CORES):
        xs = x[i * TOK_PER:(i + 1) * TOK_PER]
        # XP[g, p, c*TOK_PER+t] = xs.T[(2g+c)*128+p, t]
        XP = _cast_in(
            xs.T.reshape(NK // 2, 2, KC, TOK_PER)
            .transpose(0, 2, 1, 3)
            .reshape(NK // 2, KC, 2 * TOK_PER)
        )
        in_maps.append({"XP": XP, "WP": WP})
    return in_maps


def gather_logits(results):
    # out [128, 1024]: axes (ph, e) x (jj, t') -> logits[e, (2*jj+ph)*512+t']
    per_core = []
    for r in results:
        o = np.asarray(r["logitsP"]).astype(np.float32).reshape(2, NE, NJ // 2, NT) * 4.0
        per_core.append(o.transpose(1, 2, 0, 3).reshape(NE, TOK_PER).T)
    return np.concatenate(per_core, axis=0)  # [TOK, NE]


def _postprocess(logits, x, W):
    m = logits.max(axis=-1, keepdims=True)
    e = np.exp(logits - m)
    scores = e / e.sum(axis=-1, keepdims=True)
    order = np.argsort(-scores, axis=-1, kind="stable")
    idx = order[:, :TOPK].astype(np.int32)

    # tokens whose top-(K+1) logit gaps are within device-matmul noise get
    # their routing re-derived from exact logits
    top_l = np.take_along_axis(logits, order[:, :TOPK + 1], axis=-1)
    gaps = top_l[:, :-1] - top_l[:, 1:]
    amb = gaps.min(axis=-1) < AMB_THRESH
    if amb.any():
        xl = np.asarray(x, np.float64)[amb]
        lg = (xl @ np.asarray(W, np.float64).T).astype(np.float32)
        m2 = lg.max(axis=-1, keepdims=True)
        e2 = np.exp(lg - m2)
        sc2 = e2 / e2.sum(axis=-1, keepdims=True)
        idx2 = np.argsort(-sc2, axis=-1, kind="stable")[:, :TOPK].astype(np.int32)
        scores[amb] = sc2
        idx[amb] = idx2

    w = np.take_along_axis(scores, idx, axis=-1).astype(np.float32)
    return w, idx


def kernel(x, W):
    from concourse import bass_utils

    if "nc" not in _CACHE:
        _CACHE["nc"] = _build()
    nc = _CACHE["nc"]

    in_maps = prepare_in_maps(x, W)
    res = bass_utils.run_bass_kernel_spmd(nc, in_maps, list(range(N_CORES)))
    logits = gather_logits(res.results)
    return _postprocess(logits, x, W)
